# revision 37
# baseline (speedup 1.0000x reference)
"""Multi-head attention (B=4, C=1024, H=1, W=1500, 16 heads) on 8 TRN2 cores.

Sharding: core = 2*b + g  (b = batch 0..3, g = head-group 0..1).
Each core computes 8 heads (512 channels) for one batch and a partial
output projection; the two partials per batch are summed on the host
(plus the constant Wo@bv + bo term, folded out on the host).

v4 design notes (on top of v3):
- Input DMA: whole-tensor weight transfers (8KB lines) and per-cb x
  transfers (3KB lines) instead of 512-col chunks (1KB lines); order
  bq, wv, x[0..7], wq, wk, wo so vT/projections ride the stream
  (aggregate ~390GB/s vs ~180 before). First exp ~40us in vs ~65us.
- exp ACT-table preload at t=0 (dummy 1-elem exp) hides the ~1.5-2.7us
  table load under the input DMA.
- Head boundary is software-pipelined: head h+1's sc(0) is emitted
  before head h's pass-B (its st buffer freed at exp(h,10)), so the
  Act engine keeps a tile in flight across the boundary; one weave
  unit stolen from the next head covers the norm-chain wait before
  sc(1, next).
- Projection weave is split into half-groups of 4 accumulating
  matmuls (~1us units) distributed per kb slot: head 0 carries
  vT[6..11] JIT, head 1 carries q/k mt1, heads 2-3 mt2, heads 4-5 mt3.
  vT[0..5] and q/k mt0 run in phase 1 under the DMA shadow.
- Softmax normalization: denominator row copied to sr (DVE), e64-basis
  matmul broadcast into dead PSUM, reciprocal_approx_fast + multiply
  at base partition 0. (Tried and rejected on HW: gpsimd
  partition_broadcast reads physical partition 0 only — its input AP
  base is ignored, unlike CoreSim; partition_all_reduce works but its
  ring pass is too slow for the norm critical chain; custom-DVE
  reciprocal at base partition 64 silently produces garbage; DVE ops
  with mismatched operand base partitions fail walrus codegen.)
- Output projection is mt-major with one full-row [128,1500] store
  per mt (3KB lines) instead of per-chunk 1KB-line stores.
"""
import numpy as np
import ml_dtypes

import concourse.bass as bass
import concourse.bass_isa as bass_isa
import concourse.mybir as mybir
import concourse.tile as tile
from concourse import bacc
from concourse.bass_utils import run_bass_kernel_spmd
from contextlib import ExitStack

N_CORES = 8
B, C, W = 4, 1024, 1500
M = 512          # channels per core
NH = 8           # heads per core
D = 64           # head dim
NKB = (W + 127) // 128   # 12 key blocks
CHUNKS = [(0, 512), (512, 512), (1024, W - 1024)]
F32 = mybir.dt.float32
BF16 = mybir.dt.bfloat16
AF = mybir.ActivationFunctionType
EXP_SHIFT = -4.0
BF16NP = ml_dtypes.bfloat16


def build_nc():
    nc = bacc.Bacc("TRN2", target_bir_lowering=False, debug=False)
    # wide DRAM layouts: row-block cb of the logical matrix sits at
    # columns cb*F..(cb+1)*F, so whole-tensor DMAs move multi-KB
    # contiguous lines per partition (近 the 358GB/s roofline)
    x_d = nc.dram_tensor("x", [128, 8 * W], BF16, kind="ExternalInput").ap()
    wqT_d = nc.dram_tensor("wqT", [128, 8 * M], BF16, kind="ExternalInput").ap()
    wkT_d = nc.dram_tensor("wkT", [128, 8 * M], BF16, kind="ExternalInput").ap()
    wvT_d = nc.dram_tensor("wvT", [128, 8 * M], BF16, kind="ExternalInput").ap()
    woT_d = nc.dram_tensor("woT", [128, 4 * C], BF16, kind="ExternalInput").ap()
    bq_d = nc.dram_tensor("bq", [M, 1], F32, kind="ExternalInput").ap()
    out_d = nc.dram_tensor("out", [C, W], BF16, kind="ExternalOutput").ap()

    with tile.TileContext(nc) as tc, ExitStack() as top:
        pp = top.enter_context(tc.tile_pool(name="persist", bufs=1))
        # ---- persistent SBUF tiles (wide, sliced per row-block) ----
        x_all = pp.tile([128, 8 * W], BF16, tag="xall", name="x_all")
        wq_all = pp.tile([128, 8 * M], BF16, tag="wqall", name="wq_all")
        wk_all = pp.tile([128, 8 * M], BF16, tag="wkall", name="wk_all")
        wv_all = pp.tile([128, 8 * M], BF16, tag="wvall", name="wv_all")
        wo_all = pp.tile([128, 4 * C], BF16, tag="woall", name="wo_all")
        x_tiles = [x_all[:, cb * W:(cb + 1) * W] for cb in range(8)]
        wq_t = [wq_all[:, cb * M:(cb + 1) * M] for cb in range(8)]
        wk_t = [wk_all[:, cb * M:(cb + 1) * M] for cb in range(8)]
        wv_t = [wv_all[:, cb * M:(cb + 1) * M] for cb in range(8)]
        wo_t = [wo_all[:, cb * C:(cb + 1) * C] for cb in range(4)]
        q_tiles = [pp.tile([128, W], BF16, tag=f"q{i}", name=f"q{i}")
                   for i in range(4)]
        # per-head k, zero-padded on the other head's 64 partitions
        kp_tiles = [pp.tile([128, W], BF16, tag=f"kp{h}", name=f"kp{h}")
                    for h in range(NH)]
        # vT tiles [128, 520]: head h at cols 65h..65h+63, ones col at 65h+64
        vT_tiles = [pp.tile([128, NH * 65], BF16, tag=f"vt{i}", name=f"vt{i}")
                    for i in range(NKB)]
        o_tiles = [pp.tile([128, W], BF16, tag=f"o{i}", name=f"o{i}")
                   for i in range(4)]
        bq_tiles = [pp.tile([128, 1], F32, tag=f"bq{i}", name=f"bq{i}")
                    for i in range(4)]
        shift_t = pp.tile([128, 1], F32, tag="shift", name="shift_t")
        # denominator broadcast: row 64 of each o chunk is copied into sr
        # (rest stays zero) and an e64-basis matmul broadcasts it across
        # partitions into dead PSUM; reciprocal+multiply run at base 0
        # (custom-DVE reciprocal at base partition 64 is broken on HW)
        e64 = pp.tile([128, 128], BF16, tag="e64", name="e64")
        sr = pp.tile([128, W], BF16, tag="sr", name="sr")
        warm = pp.tile([1, 1], F32, tag="warm", name="warm")

        # ---- DMA: few wide transfers; consumers ride the stream ----
        for i in range(4):
            nc.sync.dma_start(bq_tiles[i][:], bq_d[i * 128:(i + 1) * 128, :])
        nc.sync.dma_start(wv_all[:], wvT_d[:])
        for cb in range(8):
            nc.sync.dma_start(x_all[:, cb * W:(cb + 1) * W],
                              x_d[:, cb * W:(cb + 1) * W])
        nc.sync.dma_start(wq_all[:], wqT_d[:])
        nc.sync.dma_start(wk_all[:], wkT_d[:])
        nc.sync.dma_start(wo_all[:], woT_d[:])

        nc.vector.memset(shift_t[:], EXP_SHIFT)
        nc.vector.memset(e64[:], 0.0)
        nc.vector.memset(e64[64:65, :], 1.0)
        nc.vector.memset(sr[:], 0.0)
        # preload the exp ACT table under the DMA shadow
        nc.scalar.activation(warm[0:1, 0:1], shift_t[0:1, 0:1], AF.Exp)
        # ones columns of the vT tiles (denominator trick)
        for kb in range(NKB):
            vt3 = vT_tiles[kb].rearrange("p (h c) -> p h c", c=65)
            nc.vector.memset(vt3[:, :, 64:65], 1.0)
        # zero the dead half of each kp tile (GpSimd: SBUF-only engine)
        for h in range(NH):
            if h % 2 == 0:
                nc.gpsimd.memset(kp_tiles[h][64:128, :], 0.0)
            else:
                nc.gpsimd.memset(kp_tiles[h][0:64, :], 0.0)

        # ---------- projection emitters (half-group weave units) ----------
        def qk_units(mt, which, c, pool):
            """Two ~1us units: 4+4 accumulating matmuls + evac on the 2nd."""
            q0, qn = CHUNKS[c]
            wts = wq_t if which == "q" else wk_t
            st = {}

            def a():
                ps = pool.tile([128, 512], F32, tag="pj",
                               name=f"pj_{which}{mt}_{c}")
                st["ps"] = ps
                for cb in range(4):
                    nc.tensor.matmul(
                        ps[:, :qn],
                        lhsT=wts[cb][:, mt * 128:(mt + 1) * 128],
                        rhs=x_tiles[cb][:, q0:q0 + qn],
                        start=(cb == 0), stop=False)

            def b():
                ps = st["ps"]
                for cb in range(4, 8):
                    nc.tensor.matmul(
                        ps[:, :qn],
                        lhsT=wts[cb][:, mt * 128:(mt + 1) * 128],
                        rhs=x_tiles[cb][:, q0:q0 + qn],
                        start=False, stop=(cb == 7))
                if which == "q":
                    nc.vector.tensor_scalar(
                        q_tiles[mt][:, q0:q0 + qn], ps[:, :qn],
                        bq_tiles[mt][:], None, mybir.AluOpType.add)
                else:
                    nc.vector.tensor_copy(kp_tiles[2 * mt][0:64, q0:q0 + qn],
                                          ps[0:64, :qn])
                    nc.vector.tensor_copy(kp_tiles[2 * mt + 1][64:128,
                                                              q0:q0 + qn],
                                          ps[64:128, :qn])
            return [a, b]

        def vt_units(kb, pool):
            """vT[kb] = x^T @ WvT for one 128-row key block, as 2 units."""
            klen = min(128, W - kb * 128)
            st = {}

            def a():
                ps = pool.tile([128, 512], F32, tag="pj", name=f"pj_v{kb}")
                st["ps"] = ps
                for cb in range(4):
                    nc.tensor.matmul(
                        ps[:klen, :],
                        lhsT=x_tiles[cb][:, kb * 128:kb * 128 + klen],
                        rhs=wv_t[cb][:],
                        start=(cb == 0), stop=False)

            def b():
                ps = st["ps"]
                for cb in range(4, 8):
                    nc.tensor.matmul(
                        ps[:klen, :],
                        lhsT=x_tiles[cb][:, kb * 128:kb * 128 + klen],
                        rhs=wv_t[cb][:],
                        start=False, stop=(cb == 7))
                vt3 = vT_tiles[kb].rearrange("p (h c) -> p h c", c=65)
                nc.vector.tensor_copy(vt3[:klen, :, 0:64], ps[:klen, :])
            return [a, b]

        # ---------- phase 1: under the input-DMA shadow ----------
        with ExitStack() as ph1:
            pj1 = ph1.enter_context(tc.tile_pool(name="pj1", bufs=3,
                                                 space="PSUM"))
            for kb in range(6):
                for u in vt_units(kb, pj1):
                    u()
            for c in range(3):
                for u in qk_units(0, "q", c, pj1):
                    u()
                for u in qk_units(0, "k", c, pj1):
                    u()

        # ---------- attention ----------
        with ExitStack() as ph2:
            pj = ph2.enter_context(tc.tile_pool(name="pj", bufs=1, space="PSUM"))
            stp = ph2.enter_context(tc.tile_pool(name="stp", bufs=2, space="PSUM"))
            opp = ph2.enter_context(tc.tile_pool(name="opp", bufs=1, space="PSUM"))
            ptp = ph2.enter_context(tc.tile_pool(name="ptp", bufs=13))
            rcp = ph2.enter_context(tc.tile_pool(name="rcp", bufs=2))
            ohsp = ph2.enter_context(tc.tile_pool(name="ohsp", bufs=2))

            # last processed head is EVEN: its normalized rows land in
            # o_tiles directly (DVE), so the output projection is not
            # gated on a trailing SBUF-shift DMA
            head_seq = [0, 1, 2, 3, 4, 5, 7, 6]

            # weave streams per processing position (PE filler, ~1us/unit)
            weave = {p: [] for p in range(NH)}
            for kb in range(6, NKB):
                weave[0] += vt_units(kb, pj)   # JIT for head 0's pv0
            for c in range(3):
                weave[1] += qk_units(1, "q", c, pj)
                weave[1] += qk_units(1, "k", c, pj)
            for c in range(3):
                weave[2 if c < 2 else 3] += qk_units(2, "q", c, pj)
                weave[2 if c < 2 else 3] += qk_units(2, "k", c, pj)
            for c in range(3):
                weave[4 if c < 2 else 5] += qk_units(3, "q", c, pj)
                weave[4 if c < 2 else 5] += qk_units(3, "k", c, pj)
            wcount = {p: len(weave[p]) for p in range(NH)}

            def run_weave(p, s):
                units, n = weave[p], wcount[p]
                lo, hi = s * n // NKB, (s + 1) * n // NKB
                for u in units[lo:hi]:
                    u()

            def steal_weave(p):
                # one unit from the NEXT position's stream, emitted in the
                # boundary to cover the norm-chain wait before sc(1, next)
                if p + 1 < NH and weave[p + 1]:
                    weave[p + 1].pop(0)()
                    wcount[p + 1] -= 1

            def emit_sc(h, kb, pts):
                """Scores for (h, kb): 3 chunk matmuls + exp -> pt."""
                klen = min(128, W - kb * 128)
                st = stp.tile([128, 1536], F32, tag="st", name=f"st{h}_{kb}")
                for c, (q0, qn) in enumerate(CHUNKS):
                    nc.tensor.matmul(
                        st[:klen, c * 512:c * 512 + qn],
                        lhsT=kp_tiles[h][:, kb * 128:kb * 128 + klen],
                        rhs=q_tiles[h // 2][:, q0:q0 + qn],
                        start=True, stop=True)
                pt = ptp.tile([128, 1536], BF16, tag="pt", name=f"pt{h}_{kb}")
                pts[kb] = pt
                nc.scalar.activation(pt[:klen, 0:W], st[:klen, 0:W],
                                     AF.Exp, bias=shift_t[:klen, :])
                return st

            pts_cur = {}
            emit_sc(0, 0, pts_cur)  # head 0, kb 0

            for p in range(NH):
                h = head_seq[p]
                ti, prow = h // 2, (h % 2) * 64
                o_ps0 = opp.tile([128, 512], F32, tag="op0", name=f"op{h}")
                pts = pts_cur
                st_last = None

                def emit_pv0(kb, h=h, o_ps0=o_ps0, pts=pts):
                    klen = min(128, W - kb * 128)
                    q0, qn = CHUNKS[0]
                    nc.tensor.matmul(
                        o_ps0[0:65, :qn],
                        lhsT=vT_tiles[kb][:klen, h * 65:h * 65 + 65],
                        rhs=pts[kb][:klen, q0:q0 + qn],
                        start=(kb == 0), stop=(kb == NKB - 1))

                for kb in range(1, NKB):
                    st_last = emit_sc(h, kb, pts)
                    run_weave(p, kb - 1)
                    emit_pv0(kb - 1)
                run_weave(p, NKB - 1)

                # ---- head boundary ----
                # next head's first score block: its st buffer freed at
                # exp(h, NKB-2), so Act keeps a tile in flight while the
                # PE runs pass-B below
                pts_next = {}
                if p + 1 < NH:
                    emit_sc(head_seq[p + 1], 0, pts_next)

                o_ps = [o_ps0,
                        st_last[0:128, 0:512],
                        st_last[0:128, 512:1024]]
                if prow == 0:
                    dst_tile = o_tiles[ti]
                else:
                    dst_tile = ohsp.tile([128, W], BF16, tag="ohs",
                                         name=f"ohs{h}")

                def emit_passB(c, kbs, h=h, o_ps=o_ps, pts=pts):
                    q0, qn = CHUNKS[c]
                    for kb in kbs:
                        klen = min(128, W - kb * 128)
                        nc.tensor.matmul(
                            o_ps[c][0:65, :qn],
                            lhsT=vT_tiles[kb][:klen, h * 65:h * 65 + 65],
                            rhs=pts[kb][:klen, q0:q0 + qn],
                            start=(kb == 0), stop=(kb == NKB - 1))

                # bc destinations: dead chunk-2 bank of st_last, then the
                # freed o_ps0 bank twice
                bc_dsts = [st_last[0:128, 1024:1536], o_ps0, o_ps0]

                def emit_norm_bc(c, o_ps=o_ps, bc_dsts=bc_dsts):
                    # denominator (ones column of vT lands the per-query
                    # sum in row 64) copied into sr row 64, broadcast
                    # across partitions via the e64 basis matmul
                    q0, qn = CHUNKS[c]
                    nc.vector.tensor_copy(sr[64:65, q0:q0 + qn],
                                          o_ps[c][64:65, :qn])
                    nc.tensor.matmul(
                        bc_dsts[c][:, :qn],
                        lhsT=e64[:],
                        rhs=sr[:, q0:q0 + qn],
                        start=True, stop=True)

                def emit_norm_mul(c, o_ps=o_ps, bc_dsts=bc_dsts,
                                  dst_tile=dst_tile, ti=ti, prow=prow):
                    q0, qn = CHUNKS[c]
                    rc = rcp.tile([128, 512], F32, tag="rc", name=f"rc{h}_{c}")
                    nc.vector.reciprocal_approx_fast(rc[0:64, :qn],
                                                     bc_dsts[c][0:64, :qn])
                    nc.vector.tensor_mul(
                        dst_tile[0:64, q0:q0 + qn],
                        o_ps[c][0:64, :qn], rc[0:64, :qn])
                    if prow != 0:
                        # per-chunk partition shift so consumers of this
                        # chunk need not wait for the whole head
                        nc.sync.dma_start(o_tiles[ti][64:128, q0:q0 + qn],
                                          dst_tile[0:64, q0:q0 + qn])

                emit_passB(1, range(0, NKB - 1))
                emit_pv0(NKB - 1)          # waits exp(NKB-1), ~aligned
                emit_passB(1, [NKB - 1])
                emit_norm_bc(0)
                emit_norm_mul(0)
                emit_passB(2, range(0, 6))
                emit_norm_bc(1)            # bc1 reuses o_ps0: freed above
                emit_passB(2, range(6, NKB))
                emit_norm_mul(1)
                emit_norm_bc(2)
                steal_weave(p)             # PE filler for the norm-chain
                emit_norm_mul(2)           # wait before sc(1, next)

                pts_cur = pts_next

        # ---------- output projection (own PSUM scope) ----------
        # mt-major: one full-row [128,1500] store per mt (3KB lines)
        with ExitStack() as ph3:
            oup = ph3.enter_context(tc.tile_pool(name="oup", bufs=4, space="PSUM"))
            osp = ph3.enter_context(tc.tile_pool(name="osp", bufs=2))
            for mt in range(8):
                ost = osp.tile([128, W], BF16, tag="ost", name=f"ost{mt}")
                for c, (q0, qn) in enumerate(CHUNKS):
                    ps = oup.tile([128, 512], F32, tag="ou", name=f"ou{mt}_{c}")
                    for cb in range(4):
                        nc.tensor.matmul(
                            ps[:, :qn],
                            lhsT=wo_t[cb][:, mt * 128:(mt + 1) * 128],
                            rhs=o_tiles[cb][:, q0:q0 + qn],
                            start=(cb == 0), stop=(cb == 3))
                    if (mt * 3 + c) % 2 == 0:
                        nc.scalar.copy(ost[:, q0:q0 + qn], ps[:, :qn])
                    else:
                        nc.vector.tensor_copy(ost[:, q0:q0 + qn], ps[:, :qn])
                    # two wide pieces per mt, the first as soon as chunks
                    # 0-1 are evacuated, so the final drain is short
                    if c == 1:
                        nc.sync.dma_start(
                            out_d[mt * 128:(mt + 1) * 128, 0:1024],
                            ost[:, 0:1024])
                    elif c == 2:
                        nc.sync.dma_start(
                            out_d[mt * 128:(mt + 1) * 128, 1024:W],
                            ost[:, 1024:W])

    nc.compile()
    return nc


_NC = None


def get_nc():
    global _NC
    if _NC is None:
        _NC = build_nc()
    return _NC


def _wide(a, nb):
    """[nb*128, F] -> [128, nb*F]: row-block i to column-block i."""
    return np.ascontiguousarray(
        np.concatenate([a[i * 128:(i + 1) * 128] for i in range(nb)], axis=1))


def make_in_maps(x, Wq, bq, Wk, Wv, Wo):
    s = np.float32((C // 16) ** -0.5)  # d^-0.5 = 0.125
    x = np.asarray(x, np.float32)
    Wq = np.asarray(Wq, np.float32)
    Wk = np.asarray(Wk, np.float32)
    Wv = np.asarray(Wv, np.float32)
    Wo = np.asarray(Wo, np.float32)
    bq = np.asarray(bq, np.float32)
    in_maps = []
    for core in range(N_CORES):
        b, g = core // 2, core % 2
        rs = slice(g * M, (g + 1) * M)
        in_maps.append({
            "x": _wide(x[b, :, 0, :], 8).astype(BF16NP),
            "wqT": _wide((Wq[rs] * s).T, 8).astype(BF16NP),
            "wkT": _wide(Wk[rs].T, 8).astype(BF16NP),
            "wvT": _wide(Wv[rs].T, 8).astype(BF16NP),
            "woT": _wide(Wo[:, rs].T, 4).astype(BF16NP),
            "bq": np.ascontiguousarray((bq[rs] * s).reshape(M, 1)),
        })
    return in_maps


def assemble(results, Wo, bv, bo):
    Wo = np.asarray(Wo, np.float32)
    bv = np.asarray(bv, np.float32)
    bo = np.asarray(bo, np.float32)
    const = (Wo @ bv + bo).astype(np.float32)[:, None]
    out = np.empty((B, C, 1, W), np.float32)
    for b in range(B):
        out[b, :, 0, :] = (results[2 * b]["out"].astype(np.float32)
                           + results[2 * b + 1]["out"].astype(np.float32)
                           + const)
    return out


def _results_sane(results):
    for r in results:
        o = r["out"].astype(np.float32)
        if not np.isfinite(o).all() or np.abs(o).max() > 2.0:
            return False
    return True


def kernel(x, Wq, bq, Wk, Wv, bv, Wo, bo):
    nc = get_nc()
    in_maps = make_in_maps(x, Wq, bq, Wk, Wv, Wo)
    res = run_bass_kernel_spmd(nc, in_maps, core_ids=list(range(N_CORES)))
    if not _results_sane(res.results):
        # very first execution of a freshly-loaded NEFF has been observed
        # to produce garbage once; one retry shields against that
        res = run_bass_kernel_spmd(nc, in_maps, core_ids=list(range(N_CORES)))
    return assemble(res.results, Wo, bv, bo)


# revision 38
# speedup vs baseline: 1.0913x; 1.0913x over previous
"""Multi-head attention (B=4, C=1024, H=1, W=1500, 16 heads) on 8 TRN2 cores.

Sharding: core = 2*b + g  (b = batch 0..3, g = head-group 0..1).
Each core computes 8 heads (512 channels) for one batch and a partial
output projection; the two partials per batch are summed on the host
(plus the constant Wo@bv + bo term, folded out on the host).

v4 design notes (on top of v3):
- Input DMA: whole-tensor weight transfers (8KB lines) and per-cb x
  transfers (3KB lines) instead of 512-col chunks (1KB lines); order
  bq, wv, x[0..7], wq, wk, wo so vT/projections ride the stream
  (aggregate ~390GB/s vs ~180 before). First exp ~40us in vs ~65us.
- exp ACT-table preload at t=0 (dummy 1-elem exp) hides the ~1.5-2.7us
  table load under the input DMA.
- Head boundary is software-pipelined: head h+1's sc(0) is emitted
  before head h's pass-B (its st buffer freed at exp(h,10)), so the
  Act engine keeps a tile in flight across the boundary; one weave
  unit stolen from the next head covers the norm-chain wait before
  sc(1, next).
- Projection weave is split into half-groups of 4 accumulating
  matmuls (~1us units) distributed per kb slot: head 0 carries
  vT[6..11] JIT, head 1 carries q/k mt1, heads 2-3 mt2, heads 4-5 mt3.
  vT[0..5] and q/k mt0 run in phase 1 under the DMA shadow.
- Softmax normalization: denominator row copied to sr (DVE), e64-basis
  matmul broadcast into dead PSUM, reciprocal_approx_fast + multiply
  at base partition 0. (Tried and rejected on HW: gpsimd
  partition_broadcast reads physical partition 0 only — its input AP
  base is ignored, unlike CoreSim; partition_all_reduce works but its
  ring pass is too slow for the norm critical chain; custom-DVE
  reciprocal at base partition 64 silently produces garbage; DVE ops
  with mismatched operand base partitions fail walrus codegen.)
- Output projection is mt-major with one full-row [128,1500] store
  per mt (3KB lines) instead of per-chunk 1KB-line stores.
"""
import numpy as np
import ml_dtypes

import concourse.bass as bass
import concourse.bass_isa as bass_isa
import concourse.mybir as mybir
import concourse.tile as tile
from concourse import bacc
from concourse.bass_utils import run_bass_kernel_spmd
from contextlib import ExitStack

N_CORES = 8
B, C, W = 4, 1024, 1500
M = 512          # channels per core
NH = 8           # heads per core
D = 64           # head dim
NKB = (W + 127) // 128   # 12 key blocks
CHUNKS = [(0, 512), (512, 512), (1024, W - 1024)]
F32 = mybir.dt.float32
BF16 = mybir.dt.bfloat16
AF = mybir.ActivationFunctionType
EXP_SHIFT = -4.0
BF16NP = ml_dtypes.bfloat16


def build_nc():
    nc = bacc.Bacc("TRN2", target_bir_lowering=False, debug=False)
    # wide DRAM layouts: row-block cb of the logical matrix sits at
    # columns cb*F..(cb+1)*F, so whole-tensor DMAs move multi-KB
    # contiguous lines per partition (近 the 358GB/s roofline)
    x_d = nc.dram_tensor("x", [128, 8 * W], BF16, kind="ExternalInput").ap()
    wqT_d = nc.dram_tensor("wqT", [128, 8 * M], BF16, kind="ExternalInput").ap()
    wkT_d = nc.dram_tensor("wkT", [128, 8 * M], BF16, kind="ExternalInput").ap()
    wvT_d = nc.dram_tensor("wvT", [128, 8 * M], BF16, kind="ExternalInput").ap()
    woT_d = nc.dram_tensor("woT", [128, 4 * C], BF16, kind="ExternalInput").ap()
    bq_d = nc.dram_tensor("bq", [M, 1], F32, kind="ExternalInput").ap()
    out_d = nc.dram_tensor("out", [C, W], BF16, kind="ExternalOutput").ap()

    with tile.TileContext(nc) as tc, ExitStack() as top:
        pp = top.enter_context(tc.tile_pool(name="persist", bufs=1))
        # ---- persistent SBUF tiles (wide, sliced per row-block) ----
        x_all = pp.tile([128, 8 * W], BF16, tag="xall", name="x_all")
        wq_all = pp.tile([128, 8 * M], BF16, tag="wqall", name="wq_all")
        wk_all = pp.tile([128, 8 * M], BF16, tag="wkall", name="wk_all")
        wv_all = pp.tile([128, 8 * M], BF16, tag="wvall", name="wv_all")
        wo_all = pp.tile([128, 4 * C], BF16, tag="woall", name="wo_all")
        x_tiles = [x_all[:, cb * W:(cb + 1) * W] for cb in range(8)]
        wq_t = [wq_all[:, cb * M:(cb + 1) * M] for cb in range(8)]
        wk_t = [wk_all[:, cb * M:(cb + 1) * M] for cb in range(8)]
        wv_t = [wv_all[:, cb * M:(cb + 1) * M] for cb in range(8)]
        wo_t = [wo_all[:, cb * C:(cb + 1) * C] for cb in range(4)]
        q_tiles = [pp.tile([128, W], BF16, tag=f"q{i}", name=f"q{i}")
                   for i in range(4)]
        # per-head k, zero-padded on the other head's 64 partitions
        kp_tiles = [pp.tile([128, W], BF16, tag=f"kp{h}", name=f"kp{h}")
                    for h in range(NH)]
        # vT tiles [128, 520]: head h at cols 65h..65h+63, ones col at 65h+64
        vT_tiles = [pp.tile([128, NH * 65], BF16, tag=f"vt{i}", name=f"vt{i}")
                    for i in range(NKB)]
        o_tiles = [pp.tile([128, W], BF16, tag=f"o{i}", name=f"o{i}")
                   for i in range(4)]
        bq_tiles = [pp.tile([128, 1], F32, tag=f"bq{i}", name=f"bq{i}")
                    for i in range(4)]
        shift_t = pp.tile([128, 1], F32, tag="shift", name="shift_t")
        # denominator broadcast: row 64 of each o chunk is copied into sr
        # (rest stays zero) and an e64-basis matmul broadcasts it across
        # partitions into dead PSUM; reciprocal+multiply run at base 0
        # (custom-DVE reciprocal at base partition 64 is broken on HW)
        e64 = pp.tile([128, 128], BF16, tag="e64", name="e64")
        sr = pp.tile([128, W], BF16, tag="sr", name="sr")
        warm = pp.tile([1, 1], F32, tag="warm", name="warm")

        # ---- DMA: few wide transfers; consumers ride the stream ----
        for i in range(4):
            nc.sync.dma_start(bq_tiles[i][:], bq_d[i * 128:(i + 1) * 128, :])
        nc.sync.dma_start(wv_all[:], wvT_d[:])
        for cb in range(8):
            nc.sync.dma_start(x_all[:, cb * W:(cb + 1) * W],
                              x_d[:, cb * W:(cb + 1) * W])
        nc.sync.dma_start(wq_all[:], wqT_d[:])
        nc.sync.dma_start(wk_all[:], wkT_d[:])
        nc.sync.dma_start(wo_all[:], woT_d[:])

        nc.vector.memset(shift_t[:], EXP_SHIFT)
        nc.vector.memset(e64[:], 0.0)
        nc.vector.memset(e64[64:65, :], 1.0)
        nc.vector.memset(sr[:], 0.0)
        # preload the exp ACT table under the DMA shadow
        nc.scalar.activation(warm[0:1, 0:1], shift_t[0:1, 0:1], AF.Exp)
        # ones columns of the vT tiles (denominator trick)
        for kb in range(NKB):
            vt3 = vT_tiles[kb].rearrange("p (h c) -> p h c", c=65)
            nc.vector.memset(vt3[:, :, 64:65], 1.0)
        # zero the dead half of each kp tile (GpSimd: SBUF-only engine)
        for h in range(NH):
            if h % 2 == 0:
                nc.gpsimd.memset(kp_tiles[h][64:128, :], 0.0)
            else:
                nc.gpsimd.memset(kp_tiles[h][0:64, :], 0.0)

        # ---------- projection emitters (half-group weave units) ----------
        def qk_units(mt, which, c, pool):
            """Two ~1us units: 4+4 accumulating matmuls + evac on the 2nd."""
            q0, qn = CHUNKS[c]
            wts = wq_t if which == "q" else wk_t
            st = {}

            def a():
                ps = pool.tile([128, 512], F32, tag="pj",
                               name=f"pj_{which}{mt}_{c}")
                st["ps"] = ps
                for cb in range(4):
                    nc.tensor.matmul(
                        ps[:, :qn],
                        lhsT=wts[cb][:, mt * 128:(mt + 1) * 128],
                        rhs=x_tiles[cb][:, q0:q0 + qn],
                        start=(cb == 0), stop=False)

            def b():
                ps = st["ps"]
                for cb in range(4, 8):
                    nc.tensor.matmul(
                        ps[:, :qn],
                        lhsT=wts[cb][:, mt * 128:(mt + 1) * 128],
                        rhs=x_tiles[cb][:, q0:q0 + qn],
                        start=False, stop=(cb == 7))
                if which == "q":
                    nc.vector.tensor_scalar(
                        q_tiles[mt][:, q0:q0 + qn], ps[:, :qn],
                        bq_tiles[mt][:], None, mybir.AluOpType.add)
                else:
                    nc.vector.tensor_copy(kp_tiles[2 * mt][0:64, q0:q0 + qn],
                                          ps[0:64, :qn])
                    nc.vector.tensor_copy(kp_tiles[2 * mt + 1][64:128,
                                                              q0:q0 + qn],
                                          ps[64:128, :qn])
            return [a, b]

        def vt_units(kb, pool):
            """vT[kb] = x^T @ WvT for one 128-row key block, as 2 units."""
            klen = min(128, W - kb * 128)
            st = {}

            def a():
                ps = pool.tile([128, 512], F32, tag="pj", name=f"pj_v{kb}")
                st["ps"] = ps
                for cb in range(4):
                    nc.tensor.matmul(
                        ps[:klen, :],
                        lhsT=x_tiles[cb][:, kb * 128:kb * 128 + klen],
                        rhs=wv_t[cb][:],
                        start=(cb == 0), stop=False)

            def b():
                ps = st["ps"]
                for cb in range(4, 8):
                    nc.tensor.matmul(
                        ps[:klen, :],
                        lhsT=x_tiles[cb][:, kb * 128:kb * 128 + klen],
                        rhs=wv_t[cb][:],
                        start=False, stop=(cb == 7))
                vt3 = vT_tiles[kb].rearrange("p (h c) -> p h c", c=65)
                nc.vector.tensor_copy(vt3[:klen, :, 0:64], ps[:klen, :])
            return [a, b]

        # ---------- phase 1: under the input-DMA shadow ----------
        with ExitStack() as ph1:
            pj1 = ph1.enter_context(tc.tile_pool(name="pj1", bufs=3,
                                                 space="PSUM"))
            for kb in range(6):
                for u in vt_units(kb, pj1):
                    u()
            for c in range(3):
                for u in qk_units(0, "q", c, pj1):
                    u()
                for u in qk_units(0, "k", c, pj1):
                    u()

        # ---------- attention ----------
        with ExitStack() as ph2:
            pj = ph2.enter_context(tc.tile_pool(name="pj", bufs=1, space="PSUM"))
            stp = ph2.enter_context(tc.tile_pool(name="stp", bufs=2, space="PSUM"))
            opp = ph2.enter_context(tc.tile_pool(name="opp", bufs=1, space="PSUM"))
            ptp = ph2.enter_context(tc.tile_pool(name="ptp", bufs=13))
            rcp = ph2.enter_context(tc.tile_pool(name="rcp", bufs=2))
            ohsp = ph2.enter_context(tc.tile_pool(name="ohsp", bufs=2))

            # last processed head is EVEN: its normalized rows land in
            # o_tiles directly (DVE), so the output projection is not
            # gated on a trailing SBUF-shift DMA
            head_seq = [0, 1, 2, 3, 4, 5, 7, 6]

            # weave streams per processing position (PE filler, ~1us/unit)
            weave = {p: [] for p in range(NH)}
            for kb in range(6, NKB):
                weave[0] += vt_units(kb, pj)   # JIT for head 0's pv0
            for c in range(3):
                weave[1] += qk_units(1, "q", c, pj)
                weave[1] += qk_units(1, "k", c, pj)
            for c in range(3):
                weave[2 if c < 2 else 3] += qk_units(2, "q", c, pj)
                weave[2 if c < 2 else 3] += qk_units(2, "k", c, pj)
            for c in range(3):
                weave[4 if c < 2 else 5] += qk_units(3, "q", c, pj)
                weave[4 if c < 2 else 5] += qk_units(3, "k", c, pj)
            wcount = {p: len(weave[p]) for p in range(NH)}

            def run_weave(p, s):
                units, n = weave[p], wcount[p]
                lo, hi = s * n // NKB, (s + 1) * n // NKB
                for u in units[lo:hi]:
                    u()

            def steal_weave(p):
                # one unit from the NEXT position's stream, emitted in the
                # boundary to cover the norm-chain wait before sc(1, next)
                if p + 1 < NH and weave[p + 1]:
                    weave[p + 1].pop(0)()
                    wcount[p + 1] -= 1

            def emit_sc(h, kb, pts):
                """Scores for (h, kb): 3 chunk matmuls + exp -> pt."""
                klen = min(128, W - kb * 128)
                st = stp.tile([128, 1536], F32, tag="st", name=f"st{h}_{kb}")
                for c, (q0, qn) in enumerate(CHUNKS):
                    nc.tensor.matmul(
                        st[:klen, c * 512:c * 512 + qn],
                        lhsT=kp_tiles[h][:, kb * 128:kb * 128 + klen],
                        rhs=q_tiles[h // 2][:, q0:q0 + qn],
                        start=True, stop=True)
                pt = ptp.tile([128, 1536], BF16, tag="pt", name=f"pt{h}_{kb}")
                pts[kb] = pt
                nc.scalar.activation(pt[:klen, 0:W], st[:klen, 0:W],
                                     AF.Exp, bias=shift_t[:klen, :])
                return st

            pts_cur = {}
            emit_sc(0, 0, pts_cur)  # head 0, kb 0

            for p in range(NH):
                h = head_seq[p]
                ti, prow = h // 2, (h % 2) * 64
                o_ps0 = opp.tile([128, 512], F32, tag="op0", name=f"op{h}")
                pts = pts_cur
                st_last = None

                def emit_pv0(kb, h=h, o_ps0=o_ps0, pts=pts):
                    klen = min(128, W - kb * 128)
                    q0, qn = CHUNKS[0]
                    nc.tensor.matmul(
                        o_ps0[0:65, :qn],
                        lhsT=vT_tiles[kb][:klen, h * 65:h * 65 + 65],
                        rhs=pts[kb][:klen, q0:q0 + qn],
                        start=(kb == 0), stop=(kb == NKB - 1))

                for kb in range(1, NKB):
                    st_last = emit_sc(h, kb, pts)
                    run_weave(p, kb - 1)
                    emit_pv0(kb - 1)
                run_weave(p, NKB - 1)

                # ---- head boundary ----
                # next head's first score block: its st buffer freed at
                # exp(h, NKB-2), so Act keeps a tile in flight while the
                # PE runs pass-B below
                pts_next = {}
                if p + 1 < NH:
                    emit_sc(head_seq[p + 1], 0, pts_next)

                o_ps = [o_ps0,
                        st_last[0:128, 0:512],
                        st_last[0:128, 512:1024]]
                if prow == 0:
                    dst_tile = o_tiles[ti]
                else:
                    dst_tile = ohsp.tile([128, W], BF16, tag="ohs",
                                         name=f"ohs{h}")

                def emit_passB(c, kbs, h=h, o_ps=o_ps, pts=pts):
                    q0, qn = CHUNKS[c]
                    for kb in kbs:
                        klen = min(128, W - kb * 128)
                        nc.tensor.matmul(
                            o_ps[c][0:65, :qn],
                            lhsT=vT_tiles[kb][:klen, h * 65:h * 65 + 65],
                            rhs=pts[kb][:klen, q0:q0 + qn],
                            start=(kb == 0), stop=(kb == NKB - 1))

                # bc destinations: dead chunk-2 bank of st_last, then the
                # freed o_ps0 bank twice
                bc_dsts = [st_last[0:128, 1024:1536], o_ps0, o_ps0]

                def emit_norm_bc(c, o_ps=o_ps, bc_dsts=bc_dsts):
                    # denominator (ones column of vT lands the per-query
                    # sum in row 64) copied into sr row 64, broadcast
                    # across partitions via the e64 basis matmul
                    q0, qn = CHUNKS[c]
                    nc.vector.tensor_copy(sr[64:65, q0:q0 + qn],
                                          o_ps[c][64:65, :qn])
                    nc.tensor.matmul(
                        bc_dsts[c][:, :qn],
                        lhsT=e64[:],
                        rhs=sr[:, q0:q0 + qn],
                        start=True, stop=True)

                def emit_norm_mul(c, o_ps=o_ps, bc_dsts=bc_dsts,
                                  dst_tile=dst_tile, ti=ti, prow=prow):
                    q0, qn = CHUNKS[c]
                    rc = rcp.tile([128, 512], F32, tag="rc", name=f"rc{h}_{c}")
                    nc.vector.reciprocal_approx_fast(rc[0:64, :qn],
                                                     bc_dsts[c][0:64, :qn])
                    nc.vector.tensor_mul(
                        dst_tile[0:64, q0:q0 + qn],
                        o_ps[c][0:64, :qn], rc[0:64, :qn])
                    if prow != 0:
                        # per-chunk partition shift so consumers of this
                        # chunk need not wait for the whole head
                        nc.sync.dma_start(o_tiles[ti][64:128, q0:q0 + qn],
                                          dst_tile[0:64, q0:q0 + qn])

                emit_passB(1, range(0, NKB - 1))
                emit_pv0(NKB - 1)          # waits exp(NKB-1), ~aligned
                emit_passB(1, [NKB - 1])
                emit_norm_bc(0)
                emit_norm_mul(0)
                emit_passB(2, range(0, 6))
                emit_norm_bc(1)            # bc1 reuses o_ps0: freed above
                emit_passB(2, range(6, NKB))
                emit_norm_mul(1)
                emit_norm_bc(2)
                steal_weave(p)             # PE filler for the norm-chain
                emit_norm_mul(2)           # wait before sc(1, next)

                pts_cur = pts_next

        # ---------- output projection (own PSUM scope) ----------
        # mt-major: one full-row [128,1500] store per mt (3KB lines)
        with ExitStack() as ph3:
            oup = ph3.enter_context(tc.tile_pool(name="oup", bufs=4, space="PSUM"))
            # one staging tile per mt: evacs never wait on store
            # completion (the ~2us HBM write-ack latency stays off the
            # critical chain)
            osp = ph3.enter_context(tc.tile_pool(name="osp", bufs=8))
            for mt in range(8):
                ost = osp.tile([128, W], BF16, tag="ost", name=f"ost{mt}")
                for c, (q0, qn) in enumerate(CHUNKS):
                    ps = oup.tile([128, 512], F32, tag="ou", name=f"ou{mt}_{c}")
                    for cb in range(4):
                        nc.tensor.matmul(
                            ps[:, :qn],
                            lhsT=wo_t[cb][:, mt * 128:(mt + 1) * 128],
                            rhs=o_tiles[cb][:, q0:q0 + qn],
                            start=(cb == 0), stop=(cb == 3))
                    if (mt * 3 + c) % 2 == 0:
                        nc.scalar.copy(ost[:, q0:q0 + qn], ps[:, :qn])
                    else:
                        nc.vector.tensor_copy(ost[:, q0:q0 + qn], ps[:, :qn])
                    # two wide pieces per mt, the first as soon as chunks
                    # 0-1 are evacuated, so the final drain is short
                    if c == 1:
                        nc.sync.dma_start(
                            out_d[mt * 128:(mt + 1) * 128, 0:1024],
                            ost[:, 0:1024])
                    elif c == 2:
                        nc.sync.dma_start(
                            out_d[mt * 128:(mt + 1) * 128, 1024:W],
                            ost[:, 1024:W])

    nc.compile()
    return nc


_NC = None


def get_nc():
    global _NC
    if _NC is None:
        _NC = build_nc()
    return _NC


def _wide(a, nb):
    """[nb*128, F] -> [128, nb*F]: row-block i to column-block i."""
    return np.ascontiguousarray(
        np.concatenate([a[i * 128:(i + 1) * 128] for i in range(nb)], axis=1))


def make_in_maps(x, Wq, bq, Wk, Wv, Wo):
    s = np.float32((C // 16) ** -0.5)  # d^-0.5 = 0.125
    x = np.asarray(x, np.float32)
    Wq = np.asarray(Wq, np.float32)
    Wk = np.asarray(Wk, np.float32)
    Wv = np.asarray(Wv, np.float32)
    Wo = np.asarray(Wo, np.float32)
    bq = np.asarray(bq, np.float32)
    in_maps = []
    for core in range(N_CORES):
        b, g = core // 2, core % 2
        rs = slice(g * M, (g + 1) * M)
        in_maps.append({
            "x": _wide(x[b, :, 0, :], 8).astype(BF16NP),
            "wqT": _wide((Wq[rs] * s).T, 8).astype(BF16NP),
            "wkT": _wide(Wk[rs].T, 8).astype(BF16NP),
            "wvT": _wide(Wv[rs].T, 8).astype(BF16NP),
            "woT": _wide(Wo[:, rs].T, 4).astype(BF16NP),
            "bq": np.ascontiguousarray((bq[rs] * s).reshape(M, 1)),
        })
    return in_maps


def assemble(results, Wo, bv, bo):
    Wo = np.asarray(Wo, np.float32)
    bv = np.asarray(bv, np.float32)
    bo = np.asarray(bo, np.float32)
    const = (Wo @ bv + bo).astype(np.float32)[:, None]
    out = np.empty((B, C, 1, W), np.float32)
    for b in range(B):
        out[b, :, 0, :] = (results[2 * b]["out"].astype(np.float32)
                           + results[2 * b + 1]["out"].astype(np.float32)
                           + const)
    return out


def _results_sane(results):
    for r in results:
        o = r["out"].astype(np.float32)
        if not np.isfinite(o).all() or np.abs(o).max() > 2.0:
            return False
    return True


def kernel(x, Wq, bq, Wk, Wv, bv, Wo, bo):
    nc = get_nc()
    in_maps = make_in_maps(x, Wq, bq, Wk, Wv, Wo)
    res = run_bass_kernel_spmd(nc, in_maps, core_ids=list(range(N_CORES)))
    if not _results_sane(res.results):
        # very first execution of a freshly-loaded NEFF has been observed
        # to produce garbage once; one retry shields against that
        res = run_bass_kernel_spmd(nc, in_maps, core_ids=list(range(N_CORES)))
    return assemble(res.results, Wo, bv, bo)


# revision 39
# speedup vs baseline: 1.0964x; 1.0047x over previous
"""Multi-head attention (B=4, C=1024, H=1, W=1500, 16 heads) on 8 TRN2 cores.

Sharding: core = 2*b + g  (b = batch 0..3, g = head-group 0..1).
Each core computes 8 heads (512 channels) for one batch and a partial
output projection; the two partials per batch are summed on the host
(plus the constant Wo@bv + bo term, folded out on the host).

v4 design notes (on top of v3):
- Input DMA: whole-tensor weight transfers (8KB lines) and per-cb x
  transfers (3KB lines) instead of 512-col chunks (1KB lines); order
  bq, wv, x[0..7], wq, wk, wo so vT/projections ride the stream
  (aggregate ~390GB/s vs ~180 before). First exp ~40us in vs ~65us.
- exp ACT-table preload at t=0 (dummy 1-elem exp) hides the ~1.5-2.7us
  table load under the input DMA.
- Head boundary is software-pipelined: head h+1's sc(0) is emitted
  before head h's pass-B (its st buffer freed at exp(h,10)), so the
  Act engine keeps a tile in flight across the boundary; one weave
  unit stolen from the next head covers the norm-chain wait before
  sc(1, next).
- Projection weave is split into half-groups of 4 accumulating
  matmuls (~1us units) distributed per kb slot: head 0 carries
  vT[6..11] JIT, head 1 carries q/k mt1, heads 2-3 mt2, heads 4-5 mt3.
  vT[0..5] and q/k mt0 run in phase 1 under the DMA shadow.
- Softmax normalization: denominator row copied to sr (DVE), e64-basis
  matmul broadcast into dead PSUM, reciprocal_approx_fast + multiply
  at base partition 0. (Tried and rejected on HW: gpsimd
  partition_broadcast reads physical partition 0 only — its input AP
  base is ignored, unlike CoreSim; partition_all_reduce works but its
  ring pass is too slow for the norm critical chain; custom-DVE
  reciprocal at base partition 64 silently produces garbage; DVE ops
  with mismatched operand base partitions fail walrus codegen.)
- Output projection is mt-major with one full-row [128,1500] store
  per mt (3KB lines) instead of per-chunk 1KB-line stores.
"""
import numpy as np
import ml_dtypes

import concourse.bass as bass
import concourse.bass_isa as bass_isa
import concourse.mybir as mybir
import concourse.tile as tile
from concourse import bacc
from concourse.bass_utils import run_bass_kernel_spmd
from contextlib import ExitStack

N_CORES = 8
B, C, W = 4, 1024, 1500
M = 512          # channels per core
NH = 8           # heads per core
D = 64           # head dim
NKB = (W + 127) // 128   # 12 key blocks
CHUNKS = [(0, 512), (512, 512), (1024, W - 1024)]
F32 = mybir.dt.float32
BF16 = mybir.dt.bfloat16
AF = mybir.ActivationFunctionType
EXP_SHIFT = -4.0
BF16NP = ml_dtypes.bfloat16


def build_nc():
    nc = bacc.Bacc("TRN2", target_bir_lowering=False, debug=False)
    # wide DRAM layouts: row-block cb of the logical matrix sits at
    # columns cb*F..(cb+1)*F, so whole-tensor DMAs move multi-KB
    # contiguous lines per partition (近 the 358GB/s roofline)
    x_d = nc.dram_tensor("x", [128, 8 * W], BF16, kind="ExternalInput").ap()
    wqT_d = nc.dram_tensor("wqT", [128, 8 * M], BF16, kind="ExternalInput").ap()
    wkT_d = nc.dram_tensor("wkT", [128, 8 * M], BF16, kind="ExternalInput").ap()
    wvT_d = nc.dram_tensor("wvT", [128, 8 * M], BF16, kind="ExternalInput").ap()
    woT_d = nc.dram_tensor("woT", [128, 4 * C], BF16, kind="ExternalInput").ap()
    bq_d = nc.dram_tensor("bq", [M, 1], F32, kind="ExternalInput").ap()
    out_d = nc.dram_tensor("out", [C, W], BF16, kind="ExternalOutput").ap()

    with tile.TileContext(nc) as tc, ExitStack() as top:
        pp = top.enter_context(tc.tile_pool(name="persist", bufs=1))
        # ---- persistent SBUF tiles (wide, sliced per row-block) ----
        x_all = pp.tile([128, 8 * W], BF16, tag="xall", name="x_all")
        wq_all = pp.tile([128, 8 * M], BF16, tag="wqall", name="wq_all")
        wk_all = pp.tile([128, 8 * M], BF16, tag="wkall", name="wk_all")
        wv_all = pp.tile([128, 8 * M], BF16, tag="wvall", name="wv_all")
        wo_all = pp.tile([128, 4 * C], BF16, tag="woall", name="wo_all")
        x_tiles = [x_all[:, cb * W:(cb + 1) * W] for cb in range(8)]
        wq_t = [wq_all[:, cb * M:(cb + 1) * M] for cb in range(8)]
        wk_t = [wk_all[:, cb * M:(cb + 1) * M] for cb in range(8)]
        wv_t = [wv_all[:, cb * M:(cb + 1) * M] for cb in range(8)]
        wo_t = [wo_all[:, cb * C:(cb + 1) * C] for cb in range(4)]
        q_tiles = [pp.tile([128, W], BF16, tag=f"q{i}", name=f"q{i}")
                   for i in range(4)]
        # per-head k, zero-padded on the other head's 64 partitions
        kp_tiles = [pp.tile([128, W], BF16, tag=f"kp{h}", name=f"kp{h}")
                    for h in range(NH)]
        # vT tiles [128, 520]: head h at cols 65h..65h+63, ones col at 65h+64
        vT_tiles = [pp.tile([128, NH * 65], BF16, tag=f"vt{i}", name=f"vt{i}")
                    for i in range(NKB)]
        o_tiles = [pp.tile([128, W], BF16, tag=f"o{i}", name=f"o{i}")
                   for i in range(4)]
        bq_tiles = [pp.tile([128, 1], F32, tag=f"bq{i}", name=f"bq{i}")
                    for i in range(4)]
        shift_t = pp.tile([128, 1], F32, tag="shift", name="shift_t")
        # denominator broadcast: row 64 of each o chunk is copied into sr
        # (rest stays zero) and an e64-basis matmul broadcasts it across
        # partitions into dead PSUM; reciprocal+multiply run at base 0
        # (custom-DVE reciprocal at base partition 64 is broken on HW)
        e64 = pp.tile([128, 128], BF16, tag="e64", name="e64")
        sr = pp.tile([128, W], BF16, tag="sr", name="sr")
        warm = pp.tile([1, 1], F32, tag="warm", name="warm")

        # ---- DMA: few wide transfers; consumers ride the stream ----
        for i in range(4):
            nc.sync.dma_start(bq_tiles[i][:], bq_d[i * 128:(i + 1) * 128, :])
        nc.sync.dma_start(wv_all[:], wvT_d[:])
        for cb in range(8):
            nc.sync.dma_start(x_all[:, cb * W:(cb + 1) * W],
                              x_d[:, cb * W:(cb + 1) * W])
        nc.sync.dma_start(wq_all[:], wqT_d[:])
        nc.sync.dma_start(wk_all[:], wkT_d[:])
        nc.sync.dma_start(wo_all[:], woT_d[:])

        nc.vector.memset(shift_t[:], EXP_SHIFT)
        nc.vector.memset(e64[:], 0.0)
        nc.vector.memset(e64[64:65, :], 1.0)
        nc.vector.memset(sr[:], 0.0)
        # preload the exp ACT table under the DMA shadow
        nc.scalar.activation(warm[0:1, 0:1], shift_t[0:1, 0:1], AF.Exp)
        # ones columns of the vT tiles (denominator trick)
        for kb in range(NKB):
            vt3 = vT_tiles[kb].rearrange("p (h c) -> p h c", c=65)
            nc.vector.memset(vt3[:, :, 64:65], 1.0)
        # zero the dead half of each kp tile (GpSimd: SBUF-only engine)
        for h in range(NH):
            if h % 2 == 0:
                nc.gpsimd.memset(kp_tiles[h][64:128, :], 0.0)
            else:
                nc.gpsimd.memset(kp_tiles[h][0:64, :], 0.0)

        # ---------- projection emitters (half-group weave units) ----------
        def qk_units(mt, which, c, pool):
            """Two ~1us units: 4+4 accumulating matmuls + evac on the 2nd."""
            q0, qn = CHUNKS[c]
            wts = wq_t if which == "q" else wk_t
            st = {}

            def a():
                ps = pool.tile([128, 512], F32, tag="pj",
                               name=f"pj_{which}{mt}_{c}")
                st["ps"] = ps
                for cb in range(4):
                    nc.tensor.matmul(
                        ps[:, :qn],
                        lhsT=wts[cb][:, mt * 128:(mt + 1) * 128],
                        rhs=x_tiles[cb][:, q0:q0 + qn],
                        start=(cb == 0), stop=False)

            def b():
                ps = st["ps"]
                for cb in range(4, 8):
                    nc.tensor.matmul(
                        ps[:, :qn],
                        lhsT=wts[cb][:, mt * 128:(mt + 1) * 128],
                        rhs=x_tiles[cb][:, q0:q0 + qn],
                        start=False, stop=(cb == 7))
                if which == "q":
                    nc.vector.tensor_scalar(
                        q_tiles[mt][:, q0:q0 + qn], ps[:, :qn],
                        bq_tiles[mt][:], None, mybir.AluOpType.add)
                else:
                    nc.vector.tensor_copy(kp_tiles[2 * mt][0:64, q0:q0 + qn],
                                          ps[0:64, :qn])
                    nc.vector.tensor_copy(kp_tiles[2 * mt + 1][64:128,
                                                              q0:q0 + qn],
                                          ps[64:128, :qn])
            return [a, b]

        def vt_units(kb, pool):
            """vT[kb] = x^T @ WvT for one 128-row key block, as 2 units."""
            klen = min(128, W - kb * 128)
            st = {}

            def a():
                ps = pool.tile([128, 512], F32, tag="pj", name=f"pj_v{kb}")
                st["ps"] = ps
                for cb in range(4):
                    nc.tensor.matmul(
                        ps[:klen, :],
                        lhsT=x_tiles[cb][:, kb * 128:kb * 128 + klen],
                        rhs=wv_t[cb][:],
                        start=(cb == 0), stop=False)

            def b():
                ps = st["ps"]
                for cb in range(4, 8):
                    nc.tensor.matmul(
                        ps[:klen, :],
                        lhsT=x_tiles[cb][:, kb * 128:kb * 128 + klen],
                        rhs=wv_t[cb][:],
                        start=False, stop=(cb == 7))
                vt3 = vT_tiles[kb].rearrange("p (h c) -> p h c", c=65)
                nc.vector.tensor_copy(vt3[:klen, :, 0:64], ps[:klen, :])
            return [a, b]

        # ---------- phase 1: under the input-DMA shadow ----------
        with ExitStack() as ph1:
            pj1 = ph1.enter_context(tc.tile_pool(name="pj1", bufs=3,
                                                 space="PSUM"))
            for kb in range(6):
                for u in vt_units(kb, pj1):
                    u()
            for c in range(3):
                for u in qk_units(0, "q", c, pj1):
                    u()
                for u in qk_units(0, "k", c, pj1):
                    u()

        # ---------- attention ----------
        with ExitStack() as ph2:
            pj = ph2.enter_context(tc.tile_pool(name="pj", bufs=1, space="PSUM"))
            stp = ph2.enter_context(tc.tile_pool(name="stp", bufs=2, space="PSUM"))
            opp = ph2.enter_context(tc.tile_pool(name="opp", bufs=1, space="PSUM"))
            ptp = ph2.enter_context(tc.tile_pool(name="ptp", bufs=13))
            rcp = ph2.enter_context(tc.tile_pool(name="rcp", bufs=2))
            ohsp = ph2.enter_context(tc.tile_pool(name="ohsp", bufs=2))

            # last processed head is EVEN: its normalized rows land in
            # o_tiles directly (DVE), so the output projection is not
            # gated on a trailing SBUF-shift DMA
            head_seq = [0, 1, 2, 3, 4, 5, 7, 6]

            # weave streams per processing position (PE filler, ~1us/unit)
            weave = {p: [] for p in range(NH)}
            for kb in range(6, NKB):
                weave[0] += vt_units(kb, pj)   # JIT for head 0's pv0
            for c in range(3):
                weave[1] += qk_units(1, "q", c, pj)
                weave[1] += qk_units(1, "k", c, pj)
            for c in range(3):
                weave[2 if c < 2 else 3] += qk_units(2, "q", c, pj)
                weave[2 if c < 2 else 3] += qk_units(2, "k", c, pj)
            for c in range(3):
                weave[4 if c < 2 else 5] += qk_units(3, "q", c, pj)
                weave[4 if c < 2 else 5] += qk_units(3, "k", c, pj)
            wcount = {p: len(weave[p]) for p in range(NH)}

            def run_weave(p, s):
                units, n = weave[p], wcount[p]
                lo, hi = s * n // NKB, (s + 1) * n // NKB
                for u in units[lo:hi]:
                    u()

            def steal_weave(p):
                # one unit from the NEXT position's stream, emitted in the
                # boundary to cover the norm-chain wait before sc(1, next)
                if p + 1 < NH and weave[p + 1]:
                    weave[p + 1].pop(0)()
                    wcount[p + 1] -= 1

            def emit_sc(h, kb, pts):
                """Scores for (h, kb): 3 chunk matmuls + exp -> pt."""
                klen = min(128, W - kb * 128)
                st = stp.tile([128, 1536], F32, tag="st", name=f"st{h}_{kb}")
                for c, (q0, qn) in enumerate(CHUNKS):
                    nc.tensor.matmul(
                        st[:klen, c * 512:c * 512 + qn],
                        lhsT=kp_tiles[h][:, kb * 128:kb * 128 + klen],
                        rhs=q_tiles[h // 2][:, q0:q0 + qn],
                        start=True, stop=True)
                pt = ptp.tile([128, 1536], BF16, tag="pt", name=f"pt{h}_{kb}")
                pts[kb] = pt
                nc.scalar.activation(pt[:klen, 0:W], st[:klen, 0:W],
                                     AF.Exp, bias=shift_t[:klen, :])
                return st

            pts_cur = {}
            emit_sc(0, 0, pts_cur)  # head 0, kb 0

            for p in range(NH):
                h = head_seq[p]
                ti, prow = h // 2, (h % 2) * 64
                o_ps0 = opp.tile([128, 512], F32, tag="op0", name=f"op{h}")
                pts = pts_cur
                st_last = None

                def emit_pv0(kb, h=h, o_ps0=o_ps0, pts=pts):
                    klen = min(128, W - kb * 128)
                    q0, qn = CHUNKS[0]
                    nc.tensor.matmul(
                        o_ps0[0:65, :qn],
                        lhsT=vT_tiles[kb][:klen, h * 65:h * 65 + 65],
                        rhs=pts[kb][:klen, q0:q0 + qn],
                        start=(kb == 0), stop=(kb == NKB - 1))

                for kb in range(1, NKB):
                    st_last = emit_sc(h, kb, pts)
                    run_weave(p, kb - 1)
                    emit_pv0(kb - 1)
                run_weave(p, NKB - 1)

                # ---- head boundary ----
                # next head's first score block: its st buffer freed at
                # exp(h, NKB-2), so Act keeps a tile in flight while the
                # PE runs pass-B below
                pts_next = {}
                if p + 1 < NH:
                    emit_sc(head_seq[p + 1], 0, pts_next)

                o_ps = [o_ps0,
                        st_last[0:128, 0:512],
                        st_last[0:128, 512:1024]]
                if prow == 0:
                    dst_tile = o_tiles[ti]
                else:
                    dst_tile = ohsp.tile([128, W], BF16, tag="ohs",
                                         name=f"ohs{h}")

                def emit_passB(c, kbs, h=h, o_ps=o_ps, pts=pts):
                    q0, qn = CHUNKS[c]
                    for kb in kbs:
                        klen = min(128, W - kb * 128)
                        nc.tensor.matmul(
                            o_ps[c][0:65, :qn],
                            lhsT=vT_tiles[kb][:klen, h * 65:h * 65 + 65],
                            rhs=pts[kb][:klen, q0:q0 + qn],
                            start=(kb == 0), stop=(kb == NKB - 1))

                # bc destinations: dead chunk-2 bank of st_last, then the
                # freed o_ps0 bank twice
                bc_dsts = [st_last[0:128, 1024:1536], o_ps0, o_ps0]

                def emit_norm_bc(c, o_ps=o_ps, bc_dsts=bc_dsts):
                    # denominator (ones column of vT lands the per-query
                    # sum in row 64) copied into sr row 64, broadcast
                    # across partitions via the e64 basis matmul
                    q0, qn = CHUNKS[c]
                    nc.vector.tensor_copy(sr[64:65, q0:q0 + qn],
                                          o_ps[c][64:65, :qn])
                    nc.tensor.matmul(
                        bc_dsts[c][:, :qn],
                        lhsT=e64[:],
                        rhs=sr[:, q0:q0 + qn],
                        start=True, stop=True)

                def emit_norm_mul(c, o_ps=o_ps, bc_dsts=bc_dsts,
                                  dst_tile=dst_tile, ti=ti, prow=prow):
                    q0, qn = CHUNKS[c]
                    rc = rcp.tile([128, 512], F32, tag="rc", name=f"rc{h}_{c}")
                    nc.vector.reciprocal_approx_fast(rc[0:64, :qn],
                                                     bc_dsts[c][0:64, :qn])
                    nc.vector.tensor_mul(
                        dst_tile[0:64, q0:q0 + qn],
                        o_ps[c][0:64, :qn], rc[0:64, :qn])
                    if prow != 0:
                        # per-chunk partition shift so consumers of this
                        # chunk need not wait for the whole head
                        nc.sync.dma_start(o_tiles[ti][64:128, q0:q0 + qn],
                                          dst_tile[0:64, q0:q0 + qn])

                emit_passB(1, range(0, NKB - 1))
                emit_pv0(NKB - 1)          # waits exp(NKB-1), ~aligned
                emit_passB(1, [NKB - 1])
                emit_norm_bc(0)
                emit_norm_mul(0)
                emit_passB(2, range(0, 6))
                emit_norm_bc(1)            # bc1 reuses o_ps0: freed above
                emit_passB(2, range(6, NKB))
                emit_norm_mul(1)
                emit_norm_bc(2)
                steal_weave(p)             # PE filler for the norm-chain
                emit_norm_mul(2)           # wait before sc(1, next)

                pts_cur = pts_next

        # ---------- output projection (own PSUM scope) ----------
        # mt-major: one full-row [128,1500] store per mt (3KB lines)
        with ExitStack() as ph3:
            oup = ph3.enter_context(tc.tile_pool(name="oup", bufs=4, space="PSUM"))
            # one staging tile per mt: evacs never wait on store
            # completion (the ~2us HBM write-ack latency stays off the
            # critical chain)
            osp = ph3.enter_context(tc.tile_pool(name="osp", bufs=8))
            # defer the first two mts' c2 groups behind four c0/c1
            # groups: chunk 2 of the last head is normalized ~2.4us
            # after its pass-B ends, and pure mt-major order would
            # stall the PE on it at the 3rd group
            order = ([(0, 0), (0, 1), (1, 0), (1, 1), (0, 2), (1, 2)]
                     + [(mt, c) for mt in range(2, 8) for c in range(3)])
            osts = {}
            for mt, c in order:
                q0, qn = CHUNKS[c]
                if mt not in osts:
                    osts[mt] = osp.tile([128, W], BF16, tag="ost",
                                        name=f"ost{mt}")
                ost = osts[mt]
                if True:
                    ps = oup.tile([128, 512], F32, tag="ou", name=f"ou{mt}_{c}")
                    for cb in range(4):
                        nc.tensor.matmul(
                            ps[:, :qn],
                            lhsT=wo_t[cb][:, mt * 128:(mt + 1) * 128],
                            rhs=o_tiles[cb][:, q0:q0 + qn],
                            start=(cb == 0), stop=(cb == 3))
                    if (mt * 3 + c) % 2 == 0:
                        nc.scalar.copy(ost[:, q0:q0 + qn], ps[:, :qn])
                    else:
                        nc.vector.tensor_copy(ost[:, q0:q0 + qn], ps[:, :qn])
                    # two wide pieces per mt, the first as soon as chunks
                    # 0-1 are evacuated, so the final drain is short
                    if c == 1:
                        nc.sync.dma_start(
                            out_d[mt * 128:(mt + 1) * 128, 0:1024],
                            ost[:, 0:1024])
                    elif c == 2:
                        nc.sync.dma_start(
                            out_d[mt * 128:(mt + 1) * 128, 1024:W],
                            ost[:, 1024:W])

    nc.compile()
    return nc


_NC = None


def get_nc():
    global _NC
    if _NC is None:
        _NC = build_nc()
    return _NC


def _wide(a, nb):
    """[nb*128, F] -> [128, nb*F]: row-block i to column-block i."""
    return np.ascontiguousarray(
        np.concatenate([a[i * 128:(i + 1) * 128] for i in range(nb)], axis=1))


def make_in_maps(x, Wq, bq, Wk, Wv, Wo):
    s = np.float32((C // 16) ** -0.5)  # d^-0.5 = 0.125
    x = np.asarray(x, np.float32)
    Wq = np.asarray(Wq, np.float32)
    Wk = np.asarray(Wk, np.float32)
    Wv = np.asarray(Wv, np.float32)
    Wo = np.asarray(Wo, np.float32)
    bq = np.asarray(bq, np.float32)
    in_maps = []
    for core in range(N_CORES):
        b, g = core // 2, core % 2
        rs = slice(g * M, (g + 1) * M)
        in_maps.append({
            "x": _wide(x[b, :, 0, :], 8).astype(BF16NP),
            "wqT": _wide((Wq[rs] * s).T, 8).astype(BF16NP),
            "wkT": _wide(Wk[rs].T, 8).astype(BF16NP),
            "wvT": _wide(Wv[rs].T, 8).astype(BF16NP),
            "woT": _wide(Wo[:, rs].T, 4).astype(BF16NP),
            "bq": np.ascontiguousarray((bq[rs] * s).reshape(M, 1)),
        })
    return in_maps


def assemble(results, Wo, bv, bo):
    Wo = np.asarray(Wo, np.float32)
    bv = np.asarray(bv, np.float32)
    bo = np.asarray(bo, np.float32)
    const = (Wo @ bv + bo).astype(np.float32)[:, None]
    out = np.empty((B, C, 1, W), np.float32)
    for b in range(B):
        out[b, :, 0, :] = (results[2 * b]["out"].astype(np.float32)
                           + results[2 * b + 1]["out"].astype(np.float32)
                           + const)
    return out


def _results_sane(results):
    for r in results:
        o = r["out"].astype(np.float32)
        if not np.isfinite(o).all() or np.abs(o).max() > 2.0:
            return False
    return True


def kernel(x, Wq, bq, Wk, Wv, bv, Wo, bo):
    nc = get_nc()
    in_maps = make_in_maps(x, Wq, bq, Wk, Wv, Wo)
    res = run_bass_kernel_spmd(nc, in_maps, core_ids=list(range(N_CORES)))
    if not _results_sane(res.results):
        # very first execution of a freshly-loaded NEFF has been observed
        # to produce garbage once; one retry shields against that
        res = run_bass_kernel_spmd(nc, in_maps, core_ids=list(range(N_CORES)))
    return assemble(res.results, Wo, bv, bo)
